# revision 1
# baseline (speedup 1.0000x reference)
"""Fused ACNet-style 5-branch conv block as a single 3x3 conv on Trainium2.

The reference computes
    out = conv3x3(x, w_square) + conv3x1(x, w_ver) + conv1x3(x, w_hor)
        + conv3x3(x, w_diag19 * eye3) + conv3x3(x, w_diag37 * antieye3)
All five branches are linear convs with identical output geometry, so they
fold into ONE effective 3x3 conv whose weight is the sum of the embedded /
masked branch weights.  The conv runs as 9 shifted matmuls (one per tap)
accumulated in PSUM, channels on the 128 SBUF partitions (C_in = C_out = 128):
    out[:, h, w] += W[kh,kw].T @ x_pad[:, h+kh, w+kw]

Input layout: spacer-packed rows — each padded row is 193 elements (192 data
+ 1 shared zero spacer).  The spacer acts as right-pad of row r AND left-pad
of row r+1, so every tap shift is a pure flat offset and each matmul's moving
operand is ONE contiguous 386-element run (2 output rows per PSUM bank).
Matmul operands are tagged float32r: full fp32 storage, reduced-precision
multiply at 1 row/cycle (4x faster than float32 mode, ~1.4e-4 rel err).

Sharding: data-parallel over batch — 16 images / 8 cores = 2 images per
core, weights replicated, no collectives.
"""

import sys

for _p in ("/opt/trn_rl_repo",):
    if _p not in sys.path:
        sys.path.insert(0, _p)

import numpy as np

import concourse.mybir as mybir
import concourse.tile as tile
from concourse import bacc
from concourse.bass_utils import run_bass_kernel_spmd

B, C, H, W = 16, 128, 192, 192
NCORES = 8
IPC = B // NCORES  # images per core
NTAP = 9
SW = W + 1  # spacer-packed row width (193)
XLEN = 1 + (H + 2) * SW + 4  # leading zero + 194 packed rows + tap margin
RB = 32  # output rows per block
MM_DT = mybir.dt.float32r


def _build(ipc, rb, mm_dt, repeat=1, xbufs=3, obufs=2, ahead=1):
    """Emit the per-core Bass program.

    The x-DMA for block k+ahead is issued before block k's compute/out-DMA
    in program order, so input prefetch never queues behind output drains.
    repeat>1 wraps the body in a For_i loop (timing harness only; the body
    is idempotent so outputs are unchanged).
    """
    nc = bacc.Bacc("TRN2", target_bir_lowering=False, debug=False)
    x_in = nc.dram_tensor(
        "x", [ipc, C, XLEN], mybir.dt.float32, kind="ExternalInput"
    ).ap()
    w_in = nc.dram_tensor(
        "w", [C, NTAP * C], mybir.dt.float32, kind="ExternalInput"
    ).ap()
    y_out = nc.dram_tensor(
        "y", [ipc, C, H, W], mybir.dt.float32, kind="ExternalOutput"
    ).ap()

    xtl = (rb + 2) * SW + 4  # x tile flat length per partition
    blocks = [(img, r0) for img in range(ipc) for r0 in range(0, H, rb)]

    with tile.TileContext(nc) as tc:
        with (
            tc.tile_pool(name="wp", bufs=1) as wpool,
            tc.tile_pool(name="xp", bufs=xbufs) as xpool,
            tc.tile_pool(name="op", bufs=obufs) as opool,
            tc.tile_pool(name="ps", bufs=8, space="PSUM") as pspool,
        ):
            # fp32r matmul operands must be produced as fp32r; a dtype-tagged
            # DMA qualifies (pure bitcast of the fp32 data).
            wt = wpool.tile([C, NTAP * C], mm_dt)
            nc.sync.dma_start(wt[:], w_in[:].bitcast(mm_dt))

            def load(img, r0):
                xt = xpool.tile([C, xtl], mm_dt, tag="xt", name=f"xt{img}_{r0}")
                base = r0 * SW
                nc.sync.dma_start(
                    xt[:], x_in[img, :, base : base + xtl].bitcast(mm_dt)
                )
                return xt

            def body():
                xts = [load(*blocks[k]) for k in range(min(ahead, len(blocks)))]
                for k, (img, r0) in enumerate(blocks):
                    if k + ahead < len(blocks):
                        xts.append(load(*blocks[k + ahead]))
                    xt = xts.pop(0)
                    ot = opool.tile([C, rb, W], mybir.dt.float32, tag="ot",
                                    name=f"ot{img}_{r0}")
                    for p in range(rb // 2):
                        ps = pspool.tile([C, 2 * SW], mybir.dt.float32,
                                         tag="ps", name=f"ps{p}")
                        for t in range(NTAP):
                            kh, kw = divmod(t, 3)
                            off = (2 * p + kh) * SW + kw
                            nc.tensor.matmul(
                                ps[:],
                                wt[:, t * C : (t + 1) * C],
                                xt[:, off : off + 2 * SW],
                                start=(t == 0),
                                stop=(t == NTAP - 1),
                            )
                        # strip the spacer columns while draining PSUM
                        eng = nc.scalar.copy if p % 2 == 0 else (
                            nc.vector.tensor_copy
                        )
                        eng(ot[:, 2 * p, :], ps[:, 0:W])
                        eng(ot[:, 2 * p + 1, :], ps[:, SW : SW + W])
                    nc.sync.dma_start(y_out[img, :, r0 : r0 + rb, :], ot[:])

            if repeat == 1:
                body()
            else:
                with tc.For_i(0, repeat, 1):
                    body()
    nc.compile()
    return nc


def _fold_weights(w_square, w_ver, w_hor, w_diag19, w_diag37):
    """Fold the 5 branches into one 3x3 weight, laid out [C_in, tap*C_out]."""
    eye = np.eye(3, dtype=np.float32)
    anti = eye[::-1, :]
    w_eff = (
        np.asarray(w_square, np.float32)
        + np.asarray(w_diag19, np.float32) * eye
        + np.asarray(w_diag37, np.float32) * anti
    )
    w_eff[:, :, :, 1] += np.asarray(w_ver, np.float32)[:, :, :, 0]
    w_eff[:, :, 1, :] += np.asarray(w_hor, np.float32)[:, :, 0, :]
    # [O, I, KH, KW] -> [I, KH, KW, O] -> [I, (KH*KW)*O]  (lhsT per tap)
    return np.ascontiguousarray(w_eff.transpose(1, 2, 3, 0).reshape(C, NTAP * C))


def _pack_x(x):
    """[B,C,H,W] -> spacer-packed flat [B,C,XLEN]."""
    xs = np.zeros((B, C, XLEN), np.float32)
    rows = xs[:, :, 1 : 1 + (H + 2) * SW].reshape(B, C, H + 2, SW)
    rows[:, :, 1 : H + 1, 0:W] = x
    return xs


_nc_cache = {}


def kernel(x, w_square, w_ver, w_hor, w_diag19, w_diag37):
    x = np.asarray(x, np.float32)
    w_host = _fold_weights(w_square, w_ver, w_hor, w_diag19, w_diag37)
    xs = _pack_x(x)

    if "nc" not in _nc_cache:
        _nc_cache["nc"] = _build(IPC, RB, MM_DT)
    nc = _nc_cache["nc"]

    in_maps = [
        {"x": np.ascontiguousarray(xs[c * IPC : (c + 1) * IPC]), "w": w_host}
        for c in range(NCORES)
    ]
    res = run_bass_kernel_spmd(nc, in_maps, list(range(NCORES)))
    return np.concatenate([res.results[c]["y"] for c in range(NCORES)], axis=0)



# revision 3
# speedup vs baseline: 1.2139x; 1.2139x over previous
"""Fused ACNet-style 5-branch conv block as a single 3x3 conv on Trainium2.

The reference computes
    out = conv3x3(x, w_square) + conv3x1(x, w_ver) + conv1x3(x, w_hor)
        + conv3x3(x, w_diag19 * eye3) + conv3x3(x, w_diag37 * antieye3)
All five branches are linear convs with identical output geometry, so they
fold into ONE effective 3x3 conv whose weight is the sum of the embedded /
masked branch weights.  The conv runs as 9 shifted matmuls (one per tap)
accumulated in PSUM, channels on the 128 SBUF partitions (C_in = C_out = 128):
    out[:, h, w] += W[kh,kw].T @ x_pad[:, h+kh, w+kw]

Input layout: spacer-packed rows — each padded row is 193 elements (192 data
+ 1 shared zero spacer).  The spacer acts as right-pad of row r AND left-pad
of row r+1, so every tap shift is a pure flat offset and each matmul's moving
operand is ONE contiguous 386-element run (2 output rows per PSUM bank).

Operands are bf16 (host-converted): enables fast weight load (FWL) so the
per-matmul 128-col weight load is ~53ns instead of ~107ns fp32, and halves
HBM traffic.  Matmuls are issued tap-major across a set of 4 PSUM banks so
consecutive matmuls share the stationary weights.  PSUM accumulates fp32;
outputs drain to bf16 and are converted back to fp32 on the host.

Sharding: data-parallel over batch — 16 images / 8 cores = 2 images per
core, weights replicated, no collectives.
"""

import sys

for _p in ("/opt/trn_rl_repo",):
    if _p not in sys.path:
        sys.path.insert(0, _p)

import numpy as np

import concourse.mybir as mybir
import concourse.tile as tile
from concourse import bacc
from concourse.bass_utils import run_bass_kernel_spmd

B, C, H, W = 16, 128, 192, 192
NCORES = 8
IPC = B // NCORES  # images per core
NTAP = 9
SW = W + 1  # spacer-packed row width (193)
XLEN = 1 + (H + 2) * SW + 4  # leading zero + 194 packed rows + tap margin
RB = 32  # output rows per block
GSET = 4  # 2-row groups per PSUM bank set (tap-major inner tile)
MM_DT = mybir.dt.bfloat16

_bf16 = None


def _np_bf16():
    global _bf16
    if _bf16 is None:
        _bf16 = mybir.dt.np(mybir.dt.bfloat16)
    return _bf16


def _build(ipc, rb, mm_dt, repeat=1, xbufs=3, obufs=2, ahead=1, gset=GSET):
    """Emit the per-core Bass program.

    The x-DMA for block k+ahead is issued before block k's compute/out-DMA
    in program order, so input prefetch never queues behind output drains.
    repeat>1 wraps the body in a For_i loop (timing harness only; the body
    is idempotent so outputs are unchanged).
    """
    nc = bacc.Bacc("TRN2", target_bir_lowering=False, debug=False)
    x_in = nc.dram_tensor(
        "x", [ipc, C, XLEN], mm_dt, kind="ExternalInput"
    ).ap()
    w_in = nc.dram_tensor(
        "w", [C, NTAP * C], mm_dt, kind="ExternalInput"
    ).ap()
    y_out = nc.dram_tensor(
        "y", [ipc, C, H, W], mm_dt, kind="ExternalOutput"
    ).ap()

    xtl = (rb + 2) * SW + 4  # x tile flat length per partition
    blocks = [(img, r0) for img in range(ipc) for r0 in range(0, H, rb)]

    with tile.TileContext(nc) as tc:
        with (
            tc.tile_pool(name="wp", bufs=1) as wpool,
            tc.tile_pool(name="xp", bufs=xbufs) as xpool,
            tc.tile_pool(name="op", bufs=obufs) as opool,
            tc.tile_pool(name="ps", bufs=8, space="PSUM") as pspool,
        ):
            wt = wpool.tile([C, NTAP * C], mm_dt)
            nc.sync.dma_start(wt[:], w_in[:])

            def load(img, r0):
                xt = xpool.tile([C, xtl], mm_dt, tag="xt", name=f"xt{img}_{r0}")
                base = r0 * SW
                nc.sync.dma_start(xt[:], x_in[img, :, base : base + xtl])
                return xt

            def body():
                xts = [load(*blocks[k]) for k in range(min(ahead, len(blocks)))]
                for k, (img, r0) in enumerate(blocks):
                    if k + ahead < len(blocks):
                        xts.append(load(*blocks[k + ahead]))
                    xt = xts.pop(0)
                    ot = opool.tile([C, rb, W], mm_dt, tag="ot",
                                    name=f"ot{img}_{r0}")
                    ngroups = rb // 2
                    for s in range(0, ngroups, gset):
                        nset = min(gset, ngroups - s)
                        pss = [
                            pspool.tile([C, 2 * SW], mybir.dt.float32,
                                        tag="ps", name=f"ps{s + i}")
                            for i in range(nset)
                        ]
                        # tap-major: consecutive matmuls share the stationary
                        # weights, so the PE re-loads weights 9x per set, not
                        # 9x per group.
                        for t in range(NTAP):
                            kh, kw = divmod(t, 3)
                            for i in range(nset):
                                p = s + i
                                off = (2 * p + kh) * SW + kw
                                nc.tensor.matmul(
                                    pss[i][:],
                                    wt[:, t * C : (t + 1) * C],
                                    xt[:, off : off + 2 * SW],
                                    start=(t == 0),
                                    stop=(t == NTAP - 1),
                                )
                        # strip the spacer columns while draining PSUM
                        for i in range(nset):
                            p = s + i
                            eng = nc.scalar.copy if i % 2 == 0 else (
                                nc.vector.tensor_copy
                            )
                            eng(ot[:, 2 * p, :], pss[i][:, 0:W])
                            eng(ot[:, 2 * p + 1, :], pss[i][:, SW : SW + W])
                    nc.sync.dma_start(y_out[img, :, r0 : r0 + rb, :], ot[:])

            if repeat == 1:
                body()
            else:
                with tc.For_i(0, repeat, 1):
                    body()
    nc.compile()
    return nc


def _fold_weights(w_square, w_ver, w_hor, w_diag19, w_diag37):
    """Fold the 5 branches into one 3x3 weight, laid out [C_in, tap*C_out]."""
    eye = np.eye(3, dtype=np.float32)
    anti = eye[::-1, :]
    w_eff = (
        np.asarray(w_square, np.float32)
        + np.asarray(w_diag19, np.float32) * eye
        + np.asarray(w_diag37, np.float32) * anti
    )
    w_eff[:, :, :, 1] += np.asarray(w_ver, np.float32)[:, :, :, 0]
    w_eff[:, :, 1, :] += np.asarray(w_hor, np.float32)[:, :, 0, :]
    # [O, I, KH, KW] -> [I, KH, KW, O] -> [I, (KH*KW)*O]  (lhsT per tap)
    w = np.ascontiguousarray(w_eff.transpose(1, 2, 3, 0).reshape(C, NTAP * C))
    return w.astype(_np_bf16())


def _pack_x(x):
    """[B,C,H,W] -> spacer-packed flat bf16 [B,C,XLEN]."""
    xs = np.zeros((B, C, XLEN), _np_bf16())
    rows = xs[:, :, 1 : 1 + (H + 2) * SW].reshape(B, C, H + 2, SW)
    rows[:, :, 1 : H + 1, 0:W] = x.astype(_np_bf16())
    return xs


_nc_cache = {}


def kernel(x, w_square, w_ver, w_hor, w_diag19, w_diag37):
    x = np.asarray(x, np.float32)
    w_host = _fold_weights(w_square, w_ver, w_hor, w_diag19, w_diag37)
    xs = _pack_x(x)

    if "nc" not in _nc_cache:
        _nc_cache["nc"] = _build(IPC, RB, MM_DT)
    nc = _nc_cache["nc"]

    in_maps = [
        {"x": np.ascontiguousarray(xs[c * IPC : (c + 1) * IPC]), "w": w_host}
        for c in range(NCORES)
    ]
    res = run_bass_kernel_spmd(nc, in_maps, list(range(NCORES)))
    out = np.concatenate([res.results[c]["y"] for c in range(NCORES)], axis=0)
    return out.astype(np.float32)


# revision 8
# speedup vs baseline: 1.2484x; 1.0285x over previous
"""Fused ACNet-style 5-branch conv block as a single 3x3 conv on Trainium2.

The reference computes
    out = conv3x3(x, w_square) + conv3x1(x, w_ver) + conv1x3(x, w_hor)
        + conv3x3(x, w_diag19 * eye3) + conv3x3(x, w_diag37 * antieye3)
All five branches are linear convs with identical output geometry, so they
fold into ONE effective 3x3 conv whose weight is the sum of the embedded /
masked branch weights.  The conv runs as 9 shifted matmuls (one per tap)
accumulated in PSUM, channels on the 128 SBUF partitions (C_in = C_out = 128):
    out[:, h, w] += W[kh,kw].T @ x_pad[:, h+kh, w+kw]

Input layout: spacer-packed rows — each padded row is 193 elements (192 data
+ 1 shared zero spacer).  The spacer acts as right-pad of row r AND left-pad
of row r+1, so every tap shift is a pure flat offset and each matmul's moving
operand is ONE contiguous 386-element run (2 output rows per PSUM bank).

Operands are bf16 (host-converted): enables fast weight load (FWL) so the
per-matmul 128-col weight load is ~53ns instead of ~107ns fp32, and halves
HBM traffic.  Matmuls are issued tap-major across a set of 4 PSUM banks so
consecutive matmuls share the stationary weights.  PSUM accumulates fp32;
outputs drain to bf16 and are converted back to fp32 on the host.

Sharding: data-parallel over batch — 16 images / 8 cores = 2 images per
core, weights replicated, no collectives.
"""

import sys

for _p in ("/opt/trn_rl_repo",):
    if _p not in sys.path:
        sys.path.insert(0, _p)

import numpy as np

import concourse.mybir as mybir
import concourse.tile as tile
from concourse import bacc
from concourse.bass_utils import run_bass_kernel_spmd
from concourse.tile_rust import add_dep_helper

B, C, H, W = 16, 128, 192, 192
NCORES = 8
IPC = B // NCORES  # images per core
NTAP = 9
SW = W + 1  # spacer-packed row width (193)
XLEN = 1 + (H + 2) * SW + 4  # leading zero + 194 packed rows + tap margin
RB = 32  # output rows per block
GSET = 4  # 2-row groups per PSUM bank set (tap-major inner tile)
MM_DT = mybir.dt.bfloat16

_bf16 = None


def _np_bf16():
    global _bf16
    if _bf16 is None:
        _bf16 = mybir.dt.np(mybir.dt.bfloat16)
    return _bf16


def _mm_noldw(nc, out, lhsT, rhs, start, stop):
    """InstMatmult with ldweights=False: uses the weights already loaded
    into the PE array by a preceding explicit nc.tensor.ldweights."""
    te = nc.tensor
    ifmap_ap = te.lower_ap(rhs.opt({0}), opt=False)
    weights_ap = te.lower_ap(lhsT.opt({0}), opt=False, for_matmul_weights=True)
    out_ap = te.lower_ap(out)
    return te.add_instruction(
        mybir.InstMatmult(
            name=te.bass.get_next_instruction_name(),
            replication_resolution=0,
            replication_shift_amnt=0,
            replication_num_rows=0,
            start_tensor_calc=start,
            stop_tensor_calc=stop,
            ins=[ifmap_ap, weights_ap],
            outs=[out_ap],
            perf_mode=None,
            is_transpose=None,
            ifmap_quant_offset=None,
            weights_quant_offset=None,
            bass_skip_group_check=False,
            tile_position=(0, 0),
            tile_size=(128, 128),
            ldweights=False,
        )
    )


def _build(ipc, rb, mm_dt, repeat=1, xbufs=3, obufs=2, ahead=1, gset=GSET):
    """Emit the per-core Bass program.

    The x-DMA for block k+ahead is issued before block k's compute/out-DMA
    in program order, so input prefetch never queues behind output drains.
    repeat>1 wraps the body in a For_i loop (timing harness only; the body
    is idempotent so outputs are unchanged).
    """
    nc = bacc.Bacc("TRN2", target_bir_lowering=False, debug=False)
    x_in = nc.dram_tensor(
        "x", [ipc, C, XLEN], mm_dt, kind="ExternalInput"
    ).ap()
    w_in = nc.dram_tensor(
        "w", [C, NTAP * C], mm_dt, kind="ExternalInput"
    ).ap()
    y_out = nc.dram_tensor(
        "y", [ipc, C, H, W], mm_dt, kind="ExternalOutput"
    ).ap()

    xtl = (rb + 2) * SW + 4  # x tile flat length per partition
    blocks = [(img, r0) for img in range(ipc) for r0 in range(0, H, rb)]

    with tile.TileContext(nc) as tc:
        with (
            tc.tile_pool(name="wp", bufs=1) as wpool,
            tc.tile_pool(name="xp", bufs=xbufs) as xpool,
            tc.tile_pool(name="op", bufs=obufs) as opool,
            tc.tile_pool(name="ps", bufs=8, space="PSUM") as pspool,
        ):
            wt = wpool.tile([C, NTAP * C], mm_dt)
            nc.sync.dma_start(wt[:], w_in[:])

            def load(img, r0):
                xt = xpool.tile([C, xtl], mm_dt, tag="xt", name=f"xt{img}_{r0}")
                base = r0 * SW
                nc.sync.dma_start(xt[:], x_in[img, :, base : base + xtl])
                return xt

            prev_pe = [None]  # last PE instruction, for the order chain

            def chain(bi):
                # Serialize the PE stream in program order: the scheduler
                # can't see the PE weight-state hazard between a standalone
                # ldweights and the non-self-loading matmuls that use it.
                if prev_pe[0] is not None:
                    add_dep_helper(bi.ins, prev_pe[0].ins, sync=False,
                                   reason="pe-weight-state order")
                prev_pe[0] = bi

            def body():
                xts = [load(*blocks[k]) for k in range(min(ahead, len(blocks)))]
                for k, (img, r0) in enumerate(blocks):
                    if k + ahead < len(blocks):
                        xts.append(load(*blocks[k + ahead]))
                    xt = xts.pop(0)
                    ot = opool.tile([C, rb, W], mm_dt, tag="ot",
                                    name=f"ot{img}_{r0}")
                    ngroups = rb // 2
                    for s in range(0, ngroups, gset):
                        nset = min(gset, ngroups - s)
                        pss = [
                            pspool.tile([C, 2, SW], mybir.dt.float32,
                                        tag="ps", name=f"ps{s + i}")
                            for i in range(nset)
                        ]
                        # tap-major with one explicit weight load per tap,
                        # amortized over the whole bank set.
                        for t in range(NTAP):
                            kh, kw = divmod(t, 3)
                            chain(nc.tensor.ldweights(
                                wt[:, t * C : (t + 1) * C]))
                            for i in range(nset):
                                p = s + i
                                off = (2 * p + kh) * SW + kw
                                chain(_mm_noldw(
                                    nc,
                                    pss[i][:],
                                    wt[:, t * C : (t + 1) * C],
                                    xt[:, off : off + 2 * SW],
                                    start=(t == 0),
                                    stop=(t == NTAP - 1),
                                ))
                        # strip the spacer columns while draining PSUM
                        # (one strided 2-row op per bank)
                        for i in range(nset):
                            p = s + i
                            eng = nc.scalar.copy if i % 2 == 0 else (
                                nc.vector.tensor_copy
                            )
                            eng(ot[:, 2 * p : 2 * p + 2, :],
                                pss[i][:, :, 0:W])
                    nc.sync.dma_start(y_out[img, :, r0 : r0 + rb, :], ot[:])

            if repeat == 1:
                body()
            else:
                with tc.For_i(0, repeat, 1):
                    body()
    nc.compile()
    return nc


def _fold_weights(w_square, w_ver, w_hor, w_diag19, w_diag37):
    """Fold the 5 branches into one 3x3 weight, laid out [C_in, tap*C_out]."""
    eye = np.eye(3, dtype=np.float32)
    anti = eye[::-1, :]
    w_eff = (
        np.asarray(w_square, np.float32)
        + np.asarray(w_diag19, np.float32) * eye
        + np.asarray(w_diag37, np.float32) * anti
    )
    w_eff[:, :, :, 1] += np.asarray(w_ver, np.float32)[:, :, :, 0]
    w_eff[:, :, 1, :] += np.asarray(w_hor, np.float32)[:, :, 0, :]
    # [O, I, KH, KW] -> [I, KH, KW, O] -> [I, (KH*KW)*O]  (lhsT per tap)
    w = np.ascontiguousarray(w_eff.transpose(1, 2, 3, 0).reshape(C, NTAP * C))
    return w.astype(_np_bf16())


def _pack_x(x):
    """[B,C,H,W] -> spacer-packed flat bf16 [B,C,XLEN]."""
    xs = np.zeros((B, C, XLEN), _np_bf16())
    rows = xs[:, :, 1 : 1 + (H + 2) * SW].reshape(B, C, H + 2, SW)
    rows[:, :, 1 : H + 1, 0:W] = x.astype(_np_bf16())
    return xs


_nc_cache = {}


def kernel(x, w_square, w_ver, w_hor, w_diag19, w_diag37):
    x = np.asarray(x, np.float32)
    w_host = _fold_weights(w_square, w_ver, w_hor, w_diag19, w_diag37)
    xs = _pack_x(x)

    if "nc" not in _nc_cache:
        _nc_cache["nc"] = _build(IPC, RB, MM_DT)
    nc = _nc_cache["nc"]

    in_maps = [
        {"x": np.ascontiguousarray(xs[c * IPC : (c + 1) * IPC]), "w": w_host}
        for c in range(NCORES)
    ]
    res = run_bass_kernel_spmd(nc, in_maps, list(range(NCORES)))
    out = np.concatenate([res.results[c]["y"] for c in range(NCORES)], axis=0)
    return out.astype(np.float32)
